# revision 1
# baseline (speedup 1.0000x reference)
"""Trainium2 Bass kernel for per-expert MoE FFN (gate/up/silu/down).

Problem shapes (hardcoded):
  expert_tokens        [2048, 2048] f32   (= E*T tokens, H hidden; sorted by expert)
  expert_tokens_count  [32] int64         (constant 64 per expert; unused)
  gate_proj            [32, 2048, 768] f32
  up_proj              [32, 2048, 768] f32
  down_proj            [32, 768, 2048] f32
  out                  [2048, 2048] f32

Sharding: expert-parallel across 8 NeuronCores - core c owns experts
[4c, 4c+4) and their token chunks (rows [256c, 256c+256)).  The
"all-to-all" of the hint is trivial here because tokens arrive already
sorted by expert, so the shard/gather happens host-side with numpy
slicing; each core computes its own tokens' outputs end to end.

The kernel is HBM-DMA bound, so everything is about the weight stream:

  - All streamed tensors are fp16 (half the HBM bytes of f32; 11-bit
    mantissa keeps end-to-end max rel err ~6e-4, far inside the 2e-2
    gate, while fp8's 4-bit mantissa would land ~3-7e-2).  Matmuls run
    fp16 x fp16 into fp32 PSUM; silu/mul stay fp32; h is rounded to
    fp16 at the PSUM->SBUF copy feeding the down matmul; y is stored
    fp16 and upcast on host.
  - Weights are relaid out on host into the exact SBUF tile layout so
    every DMA line is contiguous per partition (24KB gate/up, 6KB
    down).  The 16 DMA engines each cost ~15-25ns fixed + bytes/27GBps
    per packet (measured 21.4 B/ns at 1.5KB, 24.5 at 6KB, 26.7 at
    24KB), so big lines lift the aggregate from ~350 to ~425GB/s.
  - gate and up are fused into one host-side tensor (wgu) so one
    HWDGE queue entry covers both; x chunks interleave with the first
    expert's wgu chunks on the sync (SP) HWDGE queue so the first
    matmul can start ~3us in.
  - ALL weight DMAs ride the sync (SP) HWDGE queue, in consumption
    order.  The sync engine runs nothing but DMAs, so no compute
    instruction can ever head-of-line block the stream (ScalarE /
    VectorE are strict-FIFO queues - a silu or PSUM-copy parked in
    front of a dma_start would stall the weight stream behind a
    compute dependency).
  - Deep SBUF pools (a full expert of gate/up lookahead, 2 experts of
    down) keep the queue busy across phase boundaries; TensorE then
    never idles >3.4us, avoiding HAM PE-clock re-throttle.
  - A second HWDGE queue measurably HURTS: two queues splitting the
    16 DMA engines degraded sustained rate from ~420 to ~330 GB/s.
  - y stores are emitted on the same sync queue after ALL weight
    entries: their semaphores fire long before the queue drains to
    them (no head-of-line risk), and the last expert's per-chunk
    stores leave only a ~2us post-last-weight-byte tail.

Per-core dataflow (4 experts, T=64 tokens each): x^T stationary
(tokens as lhsT, so the TensorE streaming dim is the 384/512-wide
weight free dim), gate/up accumulated over 16 K-tiles into 4 PSUM
banks, silu(g)*u on ScalarE/VectorE, h^T via 6 TensorE transposes,
down accumulated over 6 K-tiles into [64, 512] PSUM chunks.
"""

import functools

import numpy as np

N_CORES = 8
E = 32                      # total experts
E_PER_CORE = E // N_CORES   # 4
T = 64                      # tokens per expert
H = 2048                    # hidden
F = 768                     # intermediate
KH = H // 128               # 16 K-tiles for gate/up
KF = F // 128               # 6 K-tiles for down
TC = E_PER_CORE * T         # 256 tokens per core
GUC = KH // 8               # 2 gate/up weight chunks per expert (8 K-tiles each)
NH = 512                    # down-proj PSUM chunk width
NHC = H // NH               # 4 down chunks per expert
XC = 2                      # x^T load chunks (8 K-tiles each)


@functools.lru_cache(maxsize=1)
def _build_nc():
    from concourse import bacc
    import concourse.mybir as mybir
    import concourse.tile as tile
    from concourse.masks import make_identity

    f32 = mybir.dt.float32
    f16 = mybir.dt.float16

    # num_devices=1: the kernel is pure SPMD with host-side sharding and
    # no collectives, so each core runs an identical single-device NEFF.
    # num_devices>1 adds a partition-id tensor + per-engine DRAM register
    # loads and branches to the preamble (measured 650-1300ns each,
    # serialized inside the startup barrier).
    nc = bacc.Bacc(
        "TRN2", target_bir_lowering=False, debug=False, num_devices=1
    )
    # Host-side layouts match SBUF tiles exactly: partition dim second-
    # to-innermost-major so each partition's line is contiguous DRAM.
    xT = nc.declare_dram_parameter("xT", [128, KH, TC], f16, isOutput=False)
    wgu = nc.declare_dram_parameter(
        "wgu", [E_PER_CORE, GUC, 128, 8, 2 * F], f16, isOutput=False
    )
    wd = nc.declare_dram_parameter(
        "wd", [E_PER_CORE, NHC, 128, KF, NH], f16, isOutput=False
    )
    out = nc.declare_dram_parameter("out", [TC, H], f16, isOutput=True)

    FH = F // 2  # 384, gate/up PSUM chunk width

    with tile.TileContext(nc) as tc:
        with (
            tc.tile_pool(name="const", bufs=1) as constp,
            tc.tile_pool(name="xt", bufs=1) as xtp,
            tc.tile_pool(name="wgup", bufs=3) as wgup,
            tc.tile_pool(name="wdp", bufs=8) as wdp,
            tc.tile_pool(name="hp", bufs=2) as hp,
            tc.tile_pool(name="ysb", bufs=2) as ysbp,
            tc.tile_pool(name="gu_ps", bufs=4, space="PSUM") as gups,
            tc.tile_pool(name="y_ps", bufs=2, space="PSUM") as yps,
            tc.tile_pool(name="ht_ps", bufs=1, space="PSUM") as htps,
            tc.tile_pool(name="warm_ps", bufs=1, space="PSUM") as warmp,
        ):
            ident = constp.tile([128, 128], f32, tag="ident")
            make_identity(nc, ident)

            # x^T resident for all 4 experts, loaded in 4 chunks
            # interleaved with the first expert's weight chunks below.
            xt = xtp.tile([128, KH, TC], f16, tag="xt")

            def load_x(c):
                nc.sync.dma_start(
                    out=xt[:, 8 * c : 8 * (c + 1), :],
                    in_=xT[:, 8 * c : 8 * (c + 1), :],
                )

            load_x(0)

            # output stores, emitted on the sync queue AFTER every weight
            # entry: their wait-semaphores fire long before the queue
            # reaches them, so they can never head-of-line block the
            # weight stream, and HWDGE drains them orders of magnitude
            # faster than GpSimd software descriptor-gen (a 0.25MB SWDGE
            # store was measured straggling ~8us past the stream's end).
            pending_outs = []

            y_pair = None
            for e in range(E_PER_CORE):
                te = e * T  # this expert's token column offset in xt

                # ---- gate/up: 4 PSUM accumulation groups over 16 K-tiles
                g0 = gups.tile([T, FH], f32, tag="gu")
                g1 = gups.tile([T, FH], f32, tag="gu")
                u0 = gups.tile([T, FH], f32, tag="gu")
                u1 = gups.tile([T, FH], f32, tag="gu")
                for c in range(GUC):
                    wgt = wgup.tile([128, 8, 2 * F], f16, tag="wgu")
                    nc.sync.dma_start(out=wgt[:], in_=wgu[e, c])
                    if e == 0 and c == 0:
                        load_x(1)
                    for kk in range(8):
                        k = 8 * c + kk
                        st = k == 0
                        sp = k == KH - 1
                        lhs = xt[:, k, te : te + T]
                        nc.tensor.matmul(
                            g0[:], lhs, wgt[:, kk, 0:FH], start=st, stop=sp
                        )
                        nc.tensor.matmul(
                            g1[:], lhs, wgt[:, kk, FH:F], start=st, stop=sp
                        )
                        nc.tensor.matmul(
                            u0[:], lhs, wgt[:, kk, F : F + FH], start=st, stop=sp
                        )
                        nc.tensor.matmul(
                            u1[:], lhs, wgt[:, kk, F + FH :], start=st, stop=sp
                        )

                # One tiny fp16 matmul at the end of each gate phase:
                # the PE executes its stream in order, so this sits right
                # after the last gate matmul and restarts the HAM idle
                # clock before the silu->transpose window (transposes
                # don't count as PE activity), keeping the first down
                # matmuls at 2.4GHz instead of the measured 634ns cold
                # starts.  fp16 only - fp32 anchors lower to LOW_HIGH
                # double-pass matmuls and disable fast-weight-load on
                # subsequent matmuls.
                warm = warmp.tile([T, T], f32, tag="warm")
                nc.tensor.matmul(
                    warm[:],
                    xt[:, 0, te : te + T],
                    xt[:, 0, te : te + T],
                    start=True,
                    stop=True,
                )

                # ---- h = silu(g) * u
                h_silu = hp.tile([T, F], f32, tag="hsilu")
                nc.scalar.activation(
                    h_silu[:, 0:FH], g0[:], mybir.ActivationFunctionType.Silu
                )
                nc.scalar.activation(
                    h_silu[:, FH:F], g1[:], mybir.ActivationFunctionType.Silu
                )
                h = hp.tile([T, F], f32, tag="h")
                nc.vector.tensor_mul(h[:, 0:FH], h_silu[:, 0:FH], u0[:])
                nc.vector.tensor_mul(h[:, FH:F], h_silu[:, FH:F], u1[:])

                # ---- h^T via TensorE transposes into one PSUM bank
                ht_ps = htps.tile([128, KF, T], f32, tag="ht")
                for c in range(KF):
                    nc.tensor.transpose(
                        ht_ps[:, c, :], h[:, 128 * c : 128 * (c + 1)], ident[:T, :T]
                    )
                hT = hp.tile([128, KF, T], f16, tag="hT")
                nc.vector.tensor_copy(out=hT[:, 0:3, :], in_=ht_ps[:, 0:3, :])
                nc.scalar.copy(out=hT[:, 3:KF, :], in_=ht_ps[:, 3:KF, :])

                # ---- down: y chunks of [64, 512] over 6 K-tiles
                if e % 2 == 0:
                    y_pair = ysbp.tile([128, H], f16, tag="ypair")
                prow = (e % 2) * T
                last_e = e == E_PER_CORE - 1
                for nh in range(NHC):
                    wdt = wdp.tile([128, KF, NH], f16, tag="wd")
                    nc.sync.dma_start(out=wdt[:], in_=wd[e, nh])
                    y_nh = yps.tile([T, NH], f32, tag="y")
                    for k in range(KF):
                        nc.tensor.matmul(
                            y_nh[:],
                            hT[:, k, :],
                            wdt[:, k, :],
                            start=(k == 0),
                            stop=(k == KF - 1),
                        )
                    # alternate PSUM->SBUF copies between ScalarE and VectorE
                    ydst = y_pair[prow : prow + T, NH * nh : NH * (nh + 1)]
                    if nh % 2 == 0:
                        nc.scalar.copy(out=ydst, in_=y_nh[:])
                    else:
                        nc.vector.tensor_copy(out=ydst, in_=y_nh[:])
                    if last_e:
                        # the final expert streams out per chunk so the
                        # post-last-weight-byte tail is one small store
                        pending_outs.append(
                            (
                                out[e * T : (e + 1) * T, NH * nh : NH * (nh + 1)],
                                ydst,
                            )
                        )

                if e % 2 == 1 and not last_e:
                    pr = (e // 2) * 2 * T
                    pending_outs.append((out[pr : pr + 2 * T, :], y_pair[:]))
                elif e == E_PER_CORE - 2:
                    # its pair partner is the streamed last expert, so this
                    # half goes out on its own as soon as its copies finish
                    pending_outs.append(
                        (out[e * T : (e + 1) * T, :], y_pair[0:T, :])
                    )

            for dst, src in pending_outs:
                nc.sync.dma_start(out=dst, in_=src)

    nc.compile()
    return nc


def _ensure_axon_hooks_stub():
    # concourse.bass_utils imports antenv.axon_hooks when tracing is
    # requested (e.g. BASS_TRACE=1 in the environment); the container's
    # antenv stub lacks that module.  Register a benign fallback so a
    # stray trace request degrades to "no profile" instead of crashing.
    import sys
    import types

    try:
        import antenv.axon_hooks  # noqa: F401
    except ImportError:
        m = types.ModuleType("antenv.axon_hooks")
        m.get_axon_ntff_profile_hook = lambda: None
        m.set_axon_ntff_profile_hook = lambda h: None
        sys.modules["antenv.axon_hooks"] = m


@functools.lru_cache(maxsize=1)
def _build_executor():
    """Pre-transferring SPMD executor.

    Like bass2jax.run_bass_via_pjrt, but inputs are device_put + blocked
    BEFORE the executable launches, so the ~300MB host->HBM upload can't
    overlap (and slow down) the kernel's own HBM streaming.
    """
    import jax
    import numpy as np
    from jax.sharding import Mesh, NamedSharding, PartitionSpec
    from jax.experimental.shard_map import shard_map
    import concourse.mybir as mybir
    from concourse import bass2jax

    nc = _build_nc()
    bass2jax.install_neuronx_cc_hook()

    partition_name = (
        nc.partition_id_tensor.name if nc.partition_id_tensor else None
    )
    in_names, out_names, out_avals, zero_shapes = [], [], [], []
    for alloc in nc.m.functions[0].allocations:
        if not isinstance(alloc, mybir.MemoryLocationSet):
            continue
        name = alloc.memorylocations[0].name
        if alloc.kind == "ExternalInput":
            if name != partition_name:
                in_names.append(name)
        elif alloc.kind == "ExternalOutput":
            shape = tuple(alloc.tensor_shape)
            dtype = mybir.dt.np(alloc.dtype)
            out_names.append(name)
            out_avals.append(jax.core.ShapedArray(shape, dtype))
            zero_shapes.append((shape, dtype))
    n_params = len(in_names)
    n_outs = len(out_avals)
    all_names = in_names + out_names + (
        [partition_name] if partition_name else []
    )

    def _body(*args):
        operands = list(args)
        if partition_name is not None:
            operands.append(bass2jax.partition_id_tensor())
        outs = bass2jax._bass_exec_p.bind(
            *operands,
            out_avals=tuple(out_avals),
            in_names=tuple(all_names),
            out_names=tuple(out_names),
            lowering_input_output_aliases=(),
            sim_require_finite=True,
            sim_require_nnan=True,
            nc=nc,
        )
        return tuple(outs)

    devices = jax.devices()[:N_CORES]
    assert len(devices) == N_CORES, f"need {N_CORES} devices, have {len(devices)}"
    mesh = Mesh(np.asarray(devices), ("core",))
    sharding = NamedSharding(mesh, PartitionSpec("core"))
    in_specs = (PartitionSpec("core"),) * (n_params + n_outs)
    out_specs = (PartitionSpec("core"),) * n_outs
    donate = tuple(range(n_params, n_params + n_outs))
    fn = jax.jit(
        shard_map(
            _body, mesh=mesh, in_specs=in_specs, out_specs=out_specs,
            check_rep=False,
        ),
        donate_argnums=donate,
        keep_unused=True,
    )

    dev_in_cache = {}

    def execute(in_maps):
        # Upload inputs once and reuse the device arrays on repeat calls
        # (e.g. warmup + traced run): re-uploading ~300MB right before
        # launch can leave residual host->HBM traffic overlapping the
        # kernel's own weight streaming.  The donated output buffers are
        # consumed by each call and must be fresh.
        key = id(in_maps)
        if key not in dev_in_cache:
            concat_in = [
                np.concatenate(
                    [in_maps[c][nm] for c in range(N_CORES)], axis=0
                )
                for nm in in_names
            ]
            dev_in_cache.clear()
            dev_in_cache[key] = [
                jax.device_put(a, sharding) for a in concat_in
            ]
        dev_in = dev_in_cache[key]
        concat_zero = [
            np.zeros((N_CORES * s[0], *s[1:]), dt) for s, dt in zero_shapes
        ]
        dev_zero = [jax.device_put(a, sharding) for a in concat_zero]
        for a in dev_in + dev_zero:
            a.block_until_ready()
        out_arrs = fn(*dev_in, *dev_zero)
        jax.block_until_ready(out_arrs)
        return [
            {
                nm: np.asarray(out_arrs[i]).reshape(
                    N_CORES, *out_avals[i].shape
                )[c]
                for i, nm in enumerate(out_names)
            }
            for c in range(N_CORES)
        ]

    return execute


def _exec(in_maps):
    """Run the SPMD kernel, returning the per-core output maps."""
    try:
        execute = _build_executor()
        return execute(in_maps)
    except Exception:
        # Fall back to the stock concourse path.
        _ensure_axon_hooks_stub()
        from concourse.bass_utils import run_bass_kernel_spmd

        nc = _build_nc()
        res = run_bass_kernel_spmd(nc, in_maps, list(range(N_CORES)))
        return res.results


def _run(in_maps, trace=False):
    _ensure_axon_hooks_stub()
    from concourse.bass_utils import run_bass_kernel_spmd

    nc = _build_nc()
    return run_bass_kernel_spmd(
        nc, in_maps, list(range(N_CORES)), trace=trace
    )


def _make_in_maps(expert_tokens, gate_proj, up_proj, down_proj):
    x = np.asarray(expert_tokens, dtype=np.float32).astype(np.float16)
    wg = np.asarray(gate_proj, dtype=np.float32).astype(np.float16)
    wu = np.asarray(up_proj, dtype=np.float32).astype(np.float16)
    wd = np.asarray(down_proj, dtype=np.float32).astype(np.float16)
    in_maps = []
    for c in range(N_CORES):
        er = slice(E_PER_CORE * c, E_PER_CORE * (c + 1))
        tr = slice(TC * c, TC * (c + 1))
        # xT[p, ko, t] = x[tr][t, 128*ko + p]
        xT = np.ascontiguousarray(
            x[tr].T.reshape(KH, 128, TC).transpose(1, 0, 2)
        )
        # wgu[e, c, p, kk, :] = concat(wg[e, 512c+128kk+p, :],
        #                              wu[e, 512c+128kk+p, :])
        wgc = wg[er].reshape(E_PER_CORE, GUC, 8, 128, F).transpose(0, 1, 3, 2, 4)
        wuc = wu[er].reshape(E_PER_CORE, GUC, 8, 128, F).transpose(0, 1, 3, 2, 4)
        wgu = np.ascontiguousarray(np.concatenate([wgc, wuc], axis=4))
        # wdl[e, nh, p, ko, hh] = wd[e, 128*ko + p, 512*nh + hh]
        wdl = np.ascontiguousarray(
            wd[er]
            .reshape(E_PER_CORE, KF, 128, NHC, NH)
            .transpose(0, 3, 2, 1, 4)
        )
        in_maps.append({"xT": xT, "wgu": wgu, "wd": wdl})
    return in_maps


def kernel(expert_tokens, expert_tokens_count, gate_proj, up_proj, down_proj):
    in_maps = _make_in_maps(expert_tokens, gate_proj, up_proj, down_proj)
    results = _exec(in_maps)
    y = np.concatenate([results[c]["out"] for c in range(N_CORES)], axis=0)
    return np.asarray(y, dtype=np.float32)



# revision 2
# speedup vs baseline: 1.0053x; 1.0053x over previous
"""Trainium2 Bass kernel for per-expert MoE FFN (gate/up/silu/down).

Problem shapes (hardcoded):
  expert_tokens        [2048, 2048] f32   (= E*T tokens, H hidden; sorted by expert)
  expert_tokens_count  [32] int64         (constant 64 per expert; unused)
  gate_proj            [32, 2048, 768] f32
  up_proj              [32, 2048, 768] f32
  down_proj            [32, 768, 2048] f32
  out                  [2048, 2048] f32

Sharding: expert-parallel across 8 NeuronCores - core c owns experts
[4c, 4c+4) and their token chunks (rows [256c, 256c+256)).  The
"all-to-all" of the hint is trivial here because tokens arrive already
sorted by expert, so the shard/gather happens host-side with numpy
slicing; each core computes its own tokens' outputs end to end.

The kernel is HBM-DMA bound (per-core weight stream ~37.75MB fp16 vs
~60us of TensorE work), so everything is about the weight stream:

  - All streamed tensors are fp16 (half the HBM bytes of f32; 11-bit
    mantissa keeps end-to-end max rel err ~6e-4, far inside the 2e-2
    gate, while fp8's 4-bit mantissa would land ~3-7e-2).  Matmuls run
    fp16 x fp16 into fp32 PSUM; silu/mul stay fp32; h is rounded to
    fp16 at the PSUM->SBUF copy feeding the down matmul; y is stored
    fp16 and upcast on host.
  - Weights are relaid out on host into the exact SBUF tile layout so
    every DMA line is contiguous per partition (24KB gate/up, 12KB
    down).  The 16 DMA engines each cost ~15-25ns fixed + bytes/27GBps
    per packet (measured 21.4 B/ns at 1.5KB, 24.5 at 6KB, 26.7 at
    24KB), so big lines lift the aggregate from ~350 to ~425GB/s.
  - gate/up weights for one expert are laid out as two COLUMN-half
    chunks (g half | u half), so the silu/mul/transpose chain for half
    0 overlaps the streaming+matmuls of half 1 instead of serializing
    after the whole expert.
  - ALL weight DMAs ride the sync (SP) HWDGE queue, in consumption
    order.  The sync engine runs nothing but DMAs, so no compute
    instruction can ever head-of-line block the stream.
  - The identity for TensorE transposes is DMA'd from DRAM instead of
    built with GpSimd make_identity: with no GpSimd instructions the
    engine drops out of the startup barrier / preamble entirely.
  - Deep SBUF pools (a full expert of gate/up lookahead, 2 experts of
    down) keep the queue busy across phase boundaries; TensorE then
    never idles >3.4us, avoiding HAM PE-clock re-throttle (the tiny
    fp16 "warm" matmul after each gate/up phase restarts the HAM idle
    clock across the silu->transpose window).
  - A second HWDGE queue measurably HURTS: two queues splitting the
    16 DMA engines degraded sustained rate from ~420 to ~330 GB/s.
  - The LAST expert is tail-critical: its final gate/up half is
    K-split into two 12KB-line chunks (so matmuls pipeline at half-
    chunk granularity - DMA completion semaphores are per dma_start),
    and its down chunks shrink [1024, 512, 256, 256] so the compute
    dependent on the last weight byte is one narrow PSUM chunk.
  - y stores are emitted on the sync queue after ALL weight entries:
    their semaphores fire long before the queue drains to them (no
    head-of-line risk); the last expert streams per-chunk stores.

Per-core dataflow (4 experts, T=64 tokens each): x^T stationary
(tokens as lhsT, so the TensorE streaming dim is the 384/512-wide
weight free dim), gate/up accumulated over 16 K-tiles into 4 PSUM
banks (two column halves x g/u), silu(g)*u on ScalarE/VectorE, h^T via
6 TensorE transposes, down accumulated over 6 K-tiles into [64, <=512]
PSUM chunks.
"""

import functools

import numpy as np

N_CORES = 8
E = 32                      # total experts
E_PER_CORE = E // N_CORES   # 4
T = 64                      # tokens per expert
H = 2048                    # hidden
F = 768                     # intermediate
KH = H // 128               # 16 K-tiles for gate/up
KF = F // 128               # 6 K-tiles for down
TC = E_PER_CORE * T         # 256 tokens per core
FH = F // 2                 # 384, gate/up PSUM chunk width
WGU_COLS = 2 * KH * F       # 24576 flat f16 cols per partition per expert
WD_COLS = KF * H            # 12288 flat f16 cols per partition per expert

# down-proj DMA chunk widths (output columns) per expert; the last
# expert tapers so the compute hanging off the last weight byte is one
# narrow chunk (6 matmuls of 256 + copy + store ~ 1.2us).
WD_SPLITS = [
    [1024, 1024],
    [1024, 1024],
    [1024, 1024],
    [1024, 512, 256, 256],
]


@functools.lru_cache(maxsize=1)
def _build_nc():
    from concourse import bacc
    import concourse.mybir as mybir
    import concourse.tile as tile

    f32 = mybir.dt.float32
    f16 = mybir.dt.float16

    # num_devices=1: the kernel is pure SPMD with host-side sharding and
    # no collectives, so each core runs an identical single-device NEFF.
    # num_devices>1 adds a partition-id tensor + per-engine DRAM register
    # loads and branches to the preamble (measured 650-1300ns each,
    # serialized inside the startup barrier).
    nc = bacc.Bacc(
        "TRN2", target_bir_lowering=False, debug=False, num_devices=1
    )
    # Host-side layouts match SBUF tiles exactly: partition dim first,
    # each partition's DMA line contiguous DRAM.
    xT = nc.declare_dram_parameter("xT", [128, KH, TC], f16, isOutput=False)
    identD = nc.declare_dram_parameter("ident", [T, T], f32, isOutput=False)
    wgu = nc.declare_dram_parameter(
        "wgu", [E_PER_CORE, 128, WGU_COLS], f16, isOutput=False
    )
    wd = nc.declare_dram_parameter(
        "wd", [E_PER_CORE, 128, WD_COLS], f16, isOutput=False
    )
    out = nc.declare_dram_parameter("out", [TC, H], f16, isOutput=True)

    with tile.TileContext(nc) as tc:
        with (
            tc.tile_pool(name="const", bufs=1) as constp,
            tc.tile_pool(name="xt", bufs=1) as xtp,
            tc.tile_pool(name="wgup", bufs=3) as wgup,
            tc.tile_pool(name="wdp", bufs=4) as wdp,
            tc.tile_pool(name="hp", bufs=2) as hp,
            tc.tile_pool(name="ysb", bufs=2) as ysbp,
            tc.tile_pool(name="gu_ps", bufs=4, space="PSUM") as gups,
            tc.tile_pool(name="y_ps", bufs=2, space="PSUM") as yps,
            tc.tile_pool(name="ht_ps", bufs=1, space="PSUM") as htps,
            tc.tile_pool(name="warm_ps", bufs=1, space="PSUM") as warmp,
        ):
            # x^T resident for all 4 experts: one 8KB-line entry.
            xt = xtp.tile([128, KH, TC], f16, tag="xt")
            nc.sync.dma_start(out=xt[:], in_=xT[:])
            ident = constp.tile([T, T], f32, tag="ident")
            nc.sync.dma_start(out=ident[:], in_=identD[:])

            # output stores, emitted on the sync queue AFTER every weight
            # entry: their wait-semaphores fire long before the queue
            # reaches them, so they can never head-of-line block the
            # weight stream.
            pending_outs = []

            y_pair = None
            for e in range(E_PER_CORE):
                te = e * T  # this expert's token column offset in xt
                last_e = e == E_PER_CORE - 1

                # ---- gate/up: two column halves, each accumulated over
                # 16 K-tiles into 2 PSUM banks (g_h, u_h).
                gu = []
                for hh in range(2):
                    gph = gups.tile([T, FH], f32, tag="gu")
                    uph = gups.tile([T, FH], f32, tag="gu")
                    gu.append((gph, uph))
                    base = hh * (KH * F)
                    if last_e and hh == 1:
                        parts = [(0, KH // 2), (KH // 2, KH)]
                    else:
                        parts = [(0, KH)]
                    for k0, k1 in parts:
                        wgt = wgup.tile([128, (k1 - k0) * F], f16, tag="wgu")
                        nc.sync.dma_start(
                            out=wgt[:],
                            in_=wgu[e, :, base + k0 * F : base + k1 * F],
                        )
                        for k in range(k0, k1):
                            off = (k - k0) * F
                            st = k == 0
                            sp = k == KH - 1
                            lhs = xt[:, k, te : te + T]
                            nc.tensor.matmul(
                                gph[:], lhs, wgt[:, off : off + FH],
                                start=st, stop=sp,
                            )
                            nc.tensor.matmul(
                                uph[:], lhs, wgt[:, off + FH : off + F],
                                start=st, stop=sp,
                            )

                # ---- h = silu(g) * u, per half (ScalarE/VectorE overlap
                # the other half's matmuls)
                h_silu = hp.tile([T, F], f32, tag="hsilu")
                h = hp.tile([T, F], f32, tag="h")
                for hh in range(2):
                    cs = hh * FH
                    nc.scalar.activation(
                        h_silu[:, cs : cs + FH], gu[hh][0][:],
                        mybir.ActivationFunctionType.Silu,
                    )
                    nc.vector.tensor_mul(
                        h[:, cs : cs + FH], h_silu[:, cs : cs + FH],
                        gu[hh][1][:],
                    )

                # One tiny fp16 matmul at the end of each gate phase:
                # the PE executes its stream in order, so this sits right
                # after the last gate matmul and restarts the HAM idle
                # clock before the silu->transpose window (transposes
                # don't count as PE activity), keeping the first down
                # matmuls at 2.4GHz instead of the measured 634ns cold
                # starts.  fp16 only - fp32 anchors lower to LOW_HIGH
                # double-pass matmuls and disable fast-weight-load on
                # subsequent matmuls.
                warm = warmp.tile([T, T], f32, tag="warm")
                nc.tensor.matmul(
                    warm[:],
                    xt[:, 0, te : te + T],
                    xt[:, 0, te : te + T],
                    start=True,
                    stop=True,
                )

                # ---- h^T via TensorE transposes into one PSUM bank
                ht_ps = htps.tile([128, KF, T], f32, tag="ht")
                for c in range(KF):
                    nc.tensor.transpose(
                        ht_ps[:, c, :], h[:, 128 * c : 128 * (c + 1)],
                        ident[:],
                    )
                hT = hp.tile([128, KF, T], f16, tag="hT")
                nc.vector.tensor_copy(out=hT[:, 0:3, :], in_=ht_ps[:, 0:3, :])
                nc.scalar.copy(out=hT[:, 3:KF, :], in_=ht_ps[:, 3:KF, :])

                # ---- down: y chunks of [64, <=512] over 6 K-tiles; DMA
                # entries carry 1024 output cols (12KB lines) feeding two
                # PSUM chunks each (tapered for the last expert).
                if e % 2 == 0:
                    y_pair = ysbp.tile([128, H], f16, tag="ypair")
                prow = (e % 2) * T
                col = 0
                ncopy = 0
                woff = 0
                for w in WD_SPLITS[e]:
                    wdt = wdp.tile([128, KF * w], f16, tag="wd")
                    nc.sync.dma_start(
                        out=wdt[:], in_=wd[e, :, woff : woff + KF * w]
                    )
                    woff += KF * w
                    for s in range(0, w, 512):
                        sw = min(512, w - s)
                        y_nh = yps.tile([T, 512], f32, tag="y")
                        for k in range(KF):
                            nc.tensor.matmul(
                                y_nh[:, 0:sw],
                                hT[:, k, :],
                                wdt[:, k * w + s : k * w + s + sw],
                                start=(k == 0),
                                stop=(k == KF - 1),
                            )
                        # alternate PSUM->SBUF copies between ScalarE and
                        # VectorE
                        ydst = y_pair[prow : prow + T, col : col + sw]
                        if ncopy % 2 == 0:
                            nc.scalar.copy(out=ydst, in_=y_nh[:, 0:sw])
                        else:
                            nc.vector.tensor_copy(out=ydst, in_=y_nh[:, 0:sw])
                        ncopy += 1
                        if last_e:
                            # the final expert streams out per chunk so
                            # the post-last-weight-byte tail is one small
                            # store
                            pending_outs.append(
                                (
                                    out[e * T : (e + 1) * T, col : col + sw],
                                    ydst,
                                )
                            )
                        col += sw

                if e % 2 == 1 and not last_e:
                    pr = (e // 2) * 2 * T
                    pending_outs.append((out[pr : pr + 2 * T, :], y_pair[:]))
                elif e == E_PER_CORE - 2:
                    # its pair partner is the streamed last expert, so this
                    # half goes out on its own as soon as its copies finish
                    pending_outs.append(
                        (out[e * T : (e + 1) * T, :], y_pair[0:T, :])
                    )

            for dst, src in pending_outs:
                nc.sync.dma_start(out=dst, in_=src)

    nc.compile()
    return nc


def _ensure_axon_hooks_stub():
    # concourse.bass_utils imports antenv.axon_hooks when tracing is
    # requested (e.g. BASS_TRACE=1 in the environment); the container's
    # antenv stub lacks that module.  Register a benign fallback so a
    # stray trace request degrades to "no profile" instead of crashing.
    import sys
    import types

    try:
        import antenv.axon_hooks  # noqa: F401
    except ImportError:
        m = types.ModuleType("antenv.axon_hooks")
        m.get_axon_ntff_profile_hook = lambda: None
        m.set_axon_ntff_profile_hook = lambda h: None
        sys.modules["antenv.axon_hooks"] = m


@functools.lru_cache(maxsize=1)
def _build_executor():
    """Pre-transferring SPMD executor.

    Like bass2jax.run_bass_via_pjrt, but inputs are device_put + blocked
    BEFORE the executable launches, so the ~300MB host->HBM upload can't
    overlap (and slow down) the kernel's own HBM streaming.
    """
    import jax
    import numpy as np
    from jax.sharding import Mesh, NamedSharding, PartitionSpec
    from jax.experimental.shard_map import shard_map
    import concourse.mybir as mybir
    from concourse import bass2jax

    nc = _build_nc()
    bass2jax.install_neuronx_cc_hook()

    partition_name = (
        nc.partition_id_tensor.name if nc.partition_id_tensor else None
    )
    in_names, out_names, out_avals, zero_shapes = [], [], [], []
    for alloc in nc.m.functions[0].allocations:
        if not isinstance(alloc, mybir.MemoryLocationSet):
            continue
        name = alloc.memorylocations[0].name
        if alloc.kind == "ExternalInput":
            if name != partition_name:
                in_names.append(name)
        elif alloc.kind == "ExternalOutput":
            shape = tuple(alloc.tensor_shape)
            dtype = mybir.dt.np(alloc.dtype)
            out_names.append(name)
            out_avals.append(jax.core.ShapedArray(shape, dtype))
            zero_shapes.append((shape, dtype))
    n_params = len(in_names)
    n_outs = len(out_avals)
    all_names = in_names + out_names + (
        [partition_name] if partition_name else []
    )

    def _body(*args):
        operands = list(args)
        if partition_name is not None:
            operands.append(bass2jax.partition_id_tensor())
        outs = bass2jax._bass_exec_p.bind(
            *operands,
            out_avals=tuple(out_avals),
            in_names=tuple(all_names),
            out_names=tuple(out_names),
            lowering_input_output_aliases=(),
            sim_require_finite=True,
            sim_require_nnan=True,
            nc=nc,
        )
        return tuple(outs)

    devices = jax.devices()[:N_CORES]
    assert len(devices) == N_CORES, f"need {N_CORES} devices, have {len(devices)}"
    mesh = Mesh(np.asarray(devices), ("core",))
    sharding = NamedSharding(mesh, PartitionSpec("core"))
    in_specs = (PartitionSpec("core"),) * (n_params + n_outs)
    out_specs = (PartitionSpec("core"),) * n_outs
    donate = tuple(range(n_params, n_params + n_outs))
    fn = jax.jit(
        shard_map(
            _body, mesh=mesh, in_specs=in_specs, out_specs=out_specs,
            check_rep=False,
        ),
        donate_argnums=donate,
        keep_unused=True,
    )

    dev_in_cache = {}

    def execute(in_maps):
        # Upload inputs once and reuse the device arrays on repeat calls
        # (e.g. warmup + traced run): re-uploading ~300MB right before
        # launch can leave residual host->HBM traffic overlapping the
        # kernel's own weight streaming.  The donated output buffers are
        # consumed by each call and must be fresh.
        key = id(in_maps)
        if key not in dev_in_cache:
            concat_in = [
                np.concatenate(
                    [in_maps[c][nm] for c in range(N_CORES)], axis=0
                )
                for nm in in_names
            ]
            dev_in_cache.clear()
            dev_in_cache[key] = [
                jax.device_put(a, sharding) for a in concat_in
            ]
        dev_in = dev_in_cache[key]
        concat_zero = [
            np.zeros((N_CORES * s[0], *s[1:]), dt) for s, dt in zero_shapes
        ]
        dev_zero = [jax.device_put(a, sharding) for a in concat_zero]
        for a in dev_in + dev_zero:
            a.block_until_ready()
        out_arrs = fn(*dev_in, *dev_zero)
        jax.block_until_ready(out_arrs)
        return [
            {
                nm: np.asarray(out_arrs[i]).reshape(
                    N_CORES, *out_avals[i].shape
                )[c]
                for i, nm in enumerate(out_names)
            }
            for c in range(N_CORES)
        ]

    return execute


def _exec(in_maps):
    """Run the SPMD kernel, returning the per-core output maps."""
    try:
        execute = _build_executor()
        return execute(in_maps)
    except Exception:
        # Fall back to the stock concourse path.
        _ensure_axon_hooks_stub()
        from concourse.bass_utils import run_bass_kernel_spmd

        nc = _build_nc()
        res = run_bass_kernel_spmd(nc, in_maps, list(range(N_CORES)))
        return res.results


def _run(in_maps, trace=False):
    _ensure_axon_hooks_stub()
    from concourse.bass_utils import run_bass_kernel_spmd

    nc = _build_nc()
    return run_bass_kernel_spmd(
        nc, in_maps, list(range(N_CORES)), trace=trace
    )


def _make_in_maps(expert_tokens, gate_proj, up_proj, down_proj):
    x = np.asarray(expert_tokens, dtype=np.float32).astype(np.float16)
    wg = np.asarray(gate_proj, dtype=np.float32).astype(np.float16)
    wu = np.asarray(up_proj, dtype=np.float32).astype(np.float16)
    wdf = np.asarray(down_proj, dtype=np.float32).astype(np.float16)
    ident = np.eye(T, dtype=np.float32)
    in_maps = []
    for c in range(N_CORES):
        er = slice(E_PER_CORE * c, E_PER_CORE * (c + 1))
        tr = slice(TC * c, TC * (c + 1))
        # xT[p, ko, t] = x[tr][t, 128*ko + p]
        xT = np.ascontiguousarray(
            x[tr].T.reshape(KH, 128, TC).transpose(1, 0, 2)
        )
        # wgu flat layout per expert/partition:
        #   [h, k, g|u, j] -> col h*12288 + k*768 + (0|384) + j
        wgc = wg[er].reshape(E_PER_CORE, KH, 128, 2, FH)  # e,k,p,h,j
        wuc = wu[er].reshape(E_PER_CORE, KH, 128, 2, FH)
        blk = np.stack([wgc, wuc], axis=4)  # e,k,p,h,gu,j
        wgu = np.ascontiguousarray(
            blk.transpose(0, 2, 3, 1, 4, 5).reshape(E_PER_CORE, 128, WGU_COLS)
        )
        # wd flat layout per expert/partition: per chunk of width w the
        # block is [k, w] (k-major), chunks concatenated.
        wdr = wdf[er].reshape(E_PER_CORE, KF, 128, H)  # e,k,p,col
        wd_rows = []
        for e in range(E_PER_CORE):
            colo = 0
            blocks = []
            for w in WD_SPLITS[e]:
                blocks.append(
                    wdr[e][:, :, colo : colo + w]
                    .transpose(1, 0, 2)
                    .reshape(128, KF * w)
                )
                colo += w
            wd_rows.append(np.concatenate(blocks, axis=1))
        wdl = np.ascontiguousarray(np.stack(wd_rows, axis=0))
        in_maps.append({"xT": xT, "ident": ident, "wgu": wgu, "wd": wdl})
    return in_maps


def kernel(expert_tokens, expert_tokens_count, gate_proj, up_proj, down_proj):
    in_maps = _make_in_maps(expert_tokens, gate_proj, up_proj, down_proj)
    results = _exec(in_maps)
    y = np.concatenate([results[c]["out"] for c in range(N_CORES)], axis=0)
    return np.asarray(y, dtype=np.float32)


# revision 4
# speedup vs baseline: 1.0202x; 1.0148x over previous
"""Trainium2 Bass kernel for per-expert MoE FFN (gate/up/silu/down).

Problem shapes (hardcoded):
  expert_tokens        [2048, 2048] f32   (= E*T tokens, H hidden; sorted by expert)
  expert_tokens_count  [32] int64         (constant 64 per expert; unused)
  gate_proj            [32, 2048, 768] f32
  up_proj              [32, 2048, 768] f32
  down_proj            [32, 768, 2048] f32
  out                  [2048, 2048] f32

Sharding: expert-parallel across 8 NeuronCores - core c owns experts
[4c, 4c+4) and their token chunks (rows [256c, 256c+256)).  The
"all-to-all" of the hint is trivial here because tokens arrive already
sorted by expert, so the shard/gather happens host-side with numpy
slicing; each core computes its own tokens' outputs end to end.

The kernel is HBM-DMA bound (per-core weight stream ~37.75MB fp16 vs
~60us of TensorE work), so everything is about the weight stream:

  - All streamed tensors are fp16 (half the HBM bytes of f32; 11-bit
    mantissa keeps end-to-end max rel err ~6e-4, far inside the 2e-2
    gate, while fp8's 4-bit mantissa would land ~3-7e-2).  Matmuls run
    fp16 x fp16 into fp32 PSUM; silu/mul stay fp32; h is rounded to
    fp16 at the PSUM->SBUF copy feeding the down matmul; y is stored
    fp16 and upcast on host.
  - Weights are relaid out on host into the exact SBUF tile layout so
    every DMA line is contiguous per partition (24KB gate/up, 12KB
    down).  The 16 DMA engines each cost ~15-25ns fixed + bytes/27GBps
    per packet (measured 21.4 B/ns at 1.5KB, 24.5 at 6KB, 26.7 at
    24KB), so big lines lift the aggregate from ~350 to ~425GB/s.
  - gate/up weights for one expert are laid out as two COLUMN-half
    chunks (g half | u half), so the silu/mul/transpose chain for half
    0 overlaps the streaming+matmuls of half 1 instead of serializing
    after the whole expert.
  - ALL weight DMAs ride the sync (SP) HWDGE queue, in consumption
    order.  The sync engine runs nothing but DMAs, so no compute
    instruction can ever head-of-line block the stream.
  - The identity for TensorE transposes is DMA'd from DRAM instead of
    built with GpSimd make_identity: with no GpSimd instructions the
    engine drops out of the startup barrier / preamble entirely.
  - Deep SBUF pools (a full expert of gate/up lookahead, 2 experts of
    down) keep the queue busy across phase boundaries; TensorE then
    never idles >3.4us, avoiding HAM PE-clock re-throttle (the tiny
    fp16 "warm" matmul after each gate/up phase restarts the HAM idle
    clock across the silu->transpose window).
  - A second HWDGE queue measurably HURTS: two queues splitting the
    16 DMA engines degraded sustained rate from ~420 to ~330 GB/s.
  - The LAST PAIR of experts is tail-critical.  Streaming order is
    [wgu e2][wgu e3][wd e2][wd e3]: expert 3's h^T is finished while
    expert 2's down weights stream, so the ~11us serial chain
    (gate/up matmuls -> silu -> transpose -> 24 down matmuls) that
    otherwise runs entirely AFTER the last weight byte instead
    overlaps the last ~15us of weight streaming.  e3's final gate/up
    half is additionally K-split into two 12KB-line chunks (DMA
    completion semaphores are per dma_start), and its down chunks
    shrink [1024, 512, 256, 256] so the compute hanging off the last
    weight byte is one narrow PSUM chunk.
  - y stores are emitted on the sync queue in consumption order but
    only behind enough later weight entries that their wait-semaphores
    fire long before the queue drains to them (no head-of-line risk);
    the last expert streams per-chunk stores at the very end.

Per-core dataflow (4 experts, T=64 tokens each): x^T stationary
(tokens as lhsT, so the TensorE streaming dim is the 384/512-wide
weight free dim), gate/up accumulated over 16 K-tiles into 4 PSUM
banks (two column halves x g/u), silu(g)*u on ScalarE/VectorE, h^T via
6 TensorE transposes, down accumulated over 6 K-tiles into [64, <=512]
PSUM chunks.
"""

import functools

import numpy as np

N_CORES = 8
E = 32                      # total experts
E_PER_CORE = E // N_CORES   # 4
T = 64                      # tokens per expert
H = 2048                    # hidden
F = 768                     # intermediate
KH = H // 128               # 16 K-tiles for gate/up
KF = F // 128               # 6 K-tiles for down
TC = E_PER_CORE * T         # 256 tokens per core
FH = F // 2                 # 384, gate/up PSUM chunk width
WGU_COLS = 2 * KH * F       # 24576 flat f16 cols per partition per expert
WD_COLS = KF * H            # 12288 flat f16 cols per partition per expert

# down-proj DMA chunk widths (output columns) per expert; the last
# expert tapers so the compute hanging off the last weight byte is one
# narrow chunk (6 matmuls of 256 + copy + store ~ 1.2us).
WD_SPLITS = [
    [1024, 1024],
    [1024, 1024],
    [1024, 1024],
    [1024, 512, 256, 256],
]


@functools.lru_cache(maxsize=1)
def _build_nc():
    from concourse import bacc
    import concourse.mybir as mybir
    import concourse.tile as tile

    f32 = mybir.dt.float32
    f16 = mybir.dt.float16

    # num_devices=1: the kernel is pure SPMD with host-side sharding and
    # no collectives, so each core runs an identical single-device NEFF.
    # num_devices>1 adds a partition-id tensor + per-engine DRAM register
    # loads and branches to the preamble (measured 650-1300ns each,
    # serialized inside the startup barrier).
    nc = bacc.Bacc(
        "TRN2", target_bir_lowering=False, debug=False, num_devices=1
    )
    # Host-side layouts match SBUF tiles exactly: partition dim first,
    # each partition's DMA line contiguous DRAM.
    xT = nc.declare_dram_parameter("xT", [128, KH, TC], f16, isOutput=False)
    identD = nc.declare_dram_parameter("ident", [T, T], f32, isOutput=False)
    wgu = nc.declare_dram_parameter(
        "wgu", [E_PER_CORE, 128, WGU_COLS], f16, isOutput=False
    )
    wd = nc.declare_dram_parameter(
        "wd", [E_PER_CORE, 128, WD_COLS], f16, isOutput=False
    )
    out = nc.declare_dram_parameter("out", [TC, H], f16, isOutput=True)

    with tile.TileContext(nc) as tc:
        with (
            tc.tile_pool(name="const", bufs=1) as constp,
            tc.tile_pool(name="xt", bufs=1) as xtp,
            tc.tile_pool(name="wgup", bufs=3) as wgup,
            tc.tile_pool(name="wdp", bufs=4) as wdp,
            tc.tile_pool(name="hp", bufs=2) as hp,
            tc.tile_pool(name="ysb", bufs=2) as ysbp,
            tc.tile_pool(name="gu_ps", bufs=4, space="PSUM") as gups,
            tc.tile_pool(name="y_ps", bufs=2, space="PSUM") as yps,
            tc.tile_pool(name="ht_ps", bufs=1, space="PSUM") as htps,
            tc.tile_pool(name="warm_ps", bufs=1, space="PSUM") as warmp,
        ):
            # x^T resident for all 4 experts: one 8KB-line entry.
            xt = xtp.tile([128, KH, TC], f16, tag="xt")
            nc.sync.dma_start(out=xt[:], in_=xT[:])
            ident = constp.tile([T, T], f32, tag="ident")
            nc.sync.dma_start(out=ident[:], in_=identD[:])

            # output stores, emitted on the sync queue AFTER every weight
            # entry: their wait-semaphores fire long before the queue
            # reaches them, so they can never head-of-line block the
            # weight stream, and moving them earlier would only push
            # weight bytes (and the compute hanging off them) later.
            pending_outs = []
            hTs = [None] * E_PER_CORE
            y_pairs = [None, None]

            def emit_gu(e):
                """Stream + compute gate/up for expert e; leaves hT[e]."""
                te = e * T  # this expert's token column offset in xt
                last_e = e == E_PER_CORE - 1
                # two column halves, each accumulated over 16 K-tiles
                # into 2 PSUM banks (g_h, u_h)
                gu = []
                for hh in range(2):
                    gph = gups.tile([T, FH], f32, tag="gu", name=f"g{e}{hh}")
                    uph = gups.tile([T, FH], f32, tag="gu", name=f"u{e}{hh}")
                    gu.append((gph, uph))
                    base = hh * (KH * F)
                    if last_e and hh == 1:
                        parts = [(0, KH // 2), (KH // 2, KH)]
                    else:
                        parts = [(0, KH)]
                    for k0, k1 in parts:
                        wgt = wgup.tile(
                            [128, (k1 - k0) * F], f16, tag="wgu",
                            name=f"wgt{e}{hh}{k0}",
                        )
                        nc.sync.dma_start(
                            out=wgt[:],
                            in_=wgu[e, :, base + k0 * F : base + k1 * F],
                        )
                        for k in range(k0, k1):
                            off = (k - k0) * F
                            st = k == 0
                            sp = k == KH - 1
                            lhs = xt[:, k, te : te + T]
                            nc.tensor.matmul(
                                gph[:], lhs, wgt[:, off : off + FH],
                                start=st, stop=sp,
                            )
                            nc.tensor.matmul(
                                uph[:], lhs, wgt[:, off + FH : off + F],
                                start=st, stop=sp,
                            )

                # h = silu(g) * u, per half (ScalarE/VectorE overlap the
                # other half's matmuls)
                h_silu = hp.tile([T, F], f32, tag="hsilu", name=f"hs{e}")
                h = hp.tile([T, F], f32, tag="h", name=f"h{e}")
                for hh in range(2):
                    cs = hh * FH
                    nc.scalar.activation(
                        h_silu[:, cs : cs + FH], gu[hh][0][:],
                        mybir.ActivationFunctionType.Silu,
                    )
                    nc.vector.tensor_mul(
                        h[:, cs : cs + FH], h_silu[:, cs : cs + FH],
                        gu[hh][1][:],
                    )

                # One tiny fp16 matmul at the end of each gate phase:
                # the PE executes its stream in order, so this sits right
                # after the last gate matmul and restarts the HAM idle
                # clock before the silu->transpose window (transposes
                # don't count as PE activity), keeping the first down
                # matmuls at 2.4GHz instead of the measured 634ns cold
                # starts.  fp16 only - fp32 anchors lower to LOW_HIGH
                # double-pass matmuls and disable fast-weight-load on
                # subsequent matmuls.
                warm = warmp.tile([T, T], f32, tag="warm", name=f"warm{e}")
                nc.tensor.matmul(
                    warm[:],
                    xt[:, 0, te : te + T],
                    xt[:, 0, te : te + T],
                    start=True,
                    stop=True,
                )

                # h^T via TensorE transposes into one PSUM bank
                ht_ps = htps.tile([128, KF, T], f32, tag="ht", name=f"htp{e}")
                for c in range(KF):
                    nc.tensor.transpose(
                        ht_ps[:, c, :], h[:, 128 * c : 128 * (c + 1)],
                        ident[:],
                    )
                hT = hp.tile([128, KF, T], f16, tag="hT", name=f"hT{e}")
                nc.vector.tensor_copy(out=hT[:, 0:3, :], in_=ht_ps[:, 0:3, :])
                nc.scalar.copy(out=hT[:, 3:KF, :], in_=ht_ps[:, 3:KF, :])
                hTs[e] = hT

            def emit_down(e):
                """Stream + compute down-proj for expert e into y_pair."""
                last_e = e == E_PER_CORE - 1
                hT = hTs[e]
                if e % 2 == 0:
                    y_pairs[e // 2] = ysbp.tile(
                        [128, H], f16, tag="ypair", name=f"yp{e // 2}"
                    )
                y_pair = y_pairs[e // 2]
                prow = (e % 2) * T
                col = 0
                ncopy = 0
                woff = 0
                for w in WD_SPLITS[e]:
                    wdt = wdp.tile(
                        [128, KF * w], f16, tag="wd", name=f"wdt{e}{col}"
                    )
                    nc.sync.dma_start(
                        out=wdt[:], in_=wd[e, :, woff : woff + KF * w]
                    )
                    woff += KF * w
                    for s in range(0, w, 512):
                        sw = min(512, w - s)
                        y_nh = yps.tile([T, 512], f32, tag="y", name=f"y{e}{col}")
                        for k in range(KF):
                            nc.tensor.matmul(
                                y_nh[:, 0:sw],
                                hT[:, k, :],
                                wdt[:, k * w + s : k * w + s + sw],
                                start=(k == 0),
                                stop=(k == KF - 1),
                            )
                        # alternate PSUM->SBUF copies between ScalarE and
                        # VectorE
                        ydst = y_pair[prow : prow + T, col : col + sw]
                        if ncopy % 2 == 0:
                            nc.scalar.copy(out=ydst, in_=y_nh[:, 0:sw])
                        else:
                            nc.vector.tensor_copy(out=ydst, in_=y_nh[:, 0:sw])
                        ncopy += 1
                        if last_e:
                            # the final expert streams out per chunk so
                            # the post-last-weight-byte tail is one small
                            # store
                            pending_outs.append(
                                (
                                    out[e * T : (e + 1) * T, col : col + sw],
                                    ydst,
                                )
                            )
                        col += sw

                if e == 1:
                    pending_outs.append((out[0 : 2 * T, :], y_pair[:]))
                elif e == E_PER_CORE - 2:
                    # its pair partner is the streamed last expert, so this
                    # half goes out on its own as soon as its copies finish
                    pending_outs.append(
                        (out[e * T : (e + 1) * T, :], y_pair[0:T, :])
                    )

            # experts 0/1: plain [wgu e][wd e] alternation.  Last pair:
            # [wgu 2][wgu 3][wd 2][wd 3] so expert 3's h^T is ready
            # before its down weights arrive and the down matmuls
            # pipeline against the final weight chunks.
            emit_gu(0)
            emit_down(0)
            emit_gu(1)
            emit_down(1)
            emit_gu(2)
            emit_gu(3)
            emit_down(2)
            emit_down(3)

            for dst, src in pending_outs:
                nc.sync.dma_start(out=dst, in_=src)

    nc.compile()
    return nc


def _ensure_axon_hooks_stub():
    # concourse.bass_utils imports antenv.axon_hooks when tracing is
    # requested (e.g. BASS_TRACE=1 in the environment); the container's
    # antenv stub lacks that module.  Register a benign fallback so a
    # stray trace request degrades to "no profile" instead of crashing.
    import sys
    import types

    try:
        import antenv.axon_hooks  # noqa: F401
    except ImportError:
        m = types.ModuleType("antenv.axon_hooks")
        m.get_axon_ntff_profile_hook = lambda: None
        m.set_axon_ntff_profile_hook = lambda h: None
        sys.modules["antenv.axon_hooks"] = m


@functools.lru_cache(maxsize=1)
def _build_executor():
    """Pre-transferring SPMD executor.

    Like bass2jax.run_bass_via_pjrt, but inputs are device_put + blocked
    BEFORE the executable launches, so the ~300MB host->HBM upload can't
    overlap (and slow down) the kernel's own HBM streaming.
    """
    import jax
    import numpy as np
    from jax.sharding import Mesh, NamedSharding, PartitionSpec
    from jax.experimental.shard_map import shard_map
    import concourse.mybir as mybir
    from concourse import bass2jax

    nc = _build_nc()
    bass2jax.install_neuronx_cc_hook()

    partition_name = (
        nc.partition_id_tensor.name if nc.partition_id_tensor else None
    )
    in_names, out_names, out_avals, zero_shapes = [], [], [], []
    for alloc in nc.m.functions[0].allocations:
        if not isinstance(alloc, mybir.MemoryLocationSet):
            continue
        name = alloc.memorylocations[0].name
        if alloc.kind == "ExternalInput":
            if name != partition_name:
                in_names.append(name)
        elif alloc.kind == "ExternalOutput":
            shape = tuple(alloc.tensor_shape)
            dtype = mybir.dt.np(alloc.dtype)
            out_names.append(name)
            out_avals.append(jax.core.ShapedArray(shape, dtype))
            zero_shapes.append((shape, dtype))
    n_params = len(in_names)
    n_outs = len(out_avals)
    all_names = in_names + out_names + (
        [partition_name] if partition_name else []
    )

    def _body(*args):
        operands = list(args)
        if partition_name is not None:
            operands.append(bass2jax.partition_id_tensor())
        outs = bass2jax._bass_exec_p.bind(
            *operands,
            out_avals=tuple(out_avals),
            in_names=tuple(all_names),
            out_names=tuple(out_names),
            lowering_input_output_aliases=(),
            sim_require_finite=True,
            sim_require_nnan=True,
            nc=nc,
        )
        return tuple(outs)

    devices = jax.devices()[:N_CORES]
    assert len(devices) == N_CORES, f"need {N_CORES} devices, have {len(devices)}"
    mesh = Mesh(np.asarray(devices), ("core",))
    sharding = NamedSharding(mesh, PartitionSpec("core"))
    in_specs = (PartitionSpec("core"),) * (n_params + n_outs)
    out_specs = (PartitionSpec("core"),) * n_outs
    donate = tuple(range(n_params, n_params + n_outs))
    fn = jax.jit(
        shard_map(
            _body, mesh=mesh, in_specs=in_specs, out_specs=out_specs,
            check_rep=False,
        ),
        donate_argnums=donate,
        keep_unused=True,
    )

    dev_in_cache = {}

    def execute(in_maps):
        # Upload inputs once and reuse the device arrays on repeat calls
        # (e.g. warmup + traced run): re-uploading ~300MB right before
        # launch can leave residual host->HBM traffic overlapping the
        # kernel's own weight streaming.  The donated output buffers are
        # consumed by each call and must be fresh.
        key = id(in_maps)
        if key not in dev_in_cache:
            concat_in = [
                np.concatenate(
                    [in_maps[c][nm] for c in range(N_CORES)], axis=0
                )
                for nm in in_names
            ]
            dev_in_cache.clear()
            dev_in_cache[key] = [
                jax.device_put(a, sharding) for a in concat_in
            ]
        dev_in = dev_in_cache[key]
        concat_zero = [
            np.zeros((N_CORES * s[0], *s[1:]), dt) for s, dt in zero_shapes
        ]
        dev_zero = [jax.device_put(a, sharding) for a in concat_zero]
        for a in dev_in + dev_zero:
            a.block_until_ready()
        out_arrs = fn(*dev_in, *dev_zero)
        jax.block_until_ready(out_arrs)
        return [
            {
                nm: np.asarray(out_arrs[i]).reshape(
                    N_CORES, *out_avals[i].shape
                )[c]
                for i, nm in enumerate(out_names)
            }
            for c in range(N_CORES)
        ]

    return execute


def _exec(in_maps):
    """Run the SPMD kernel, returning the per-core output maps."""
    try:
        execute = _build_executor()
        return execute(in_maps)
    except Exception:
        # Fall back to the stock concourse path.
        _ensure_axon_hooks_stub()
        from concourse.bass_utils import run_bass_kernel_spmd

        nc = _build_nc()
        res = run_bass_kernel_spmd(nc, in_maps, list(range(N_CORES)))
        return res.results


def _run(in_maps, trace=False):
    _ensure_axon_hooks_stub()
    from concourse.bass_utils import run_bass_kernel_spmd

    nc = _build_nc()
    return run_bass_kernel_spmd(
        nc, in_maps, list(range(N_CORES)), trace=trace
    )


def _make_in_maps(expert_tokens, gate_proj, up_proj, down_proj):
    x = np.asarray(expert_tokens, dtype=np.float32).astype(np.float16)
    wg = np.asarray(gate_proj, dtype=np.float32).astype(np.float16)
    wu = np.asarray(up_proj, dtype=np.float32).astype(np.float16)
    wdf = np.asarray(down_proj, dtype=np.float32).astype(np.float16)
    ident = np.eye(T, dtype=np.float32)
    in_maps = []
    for c in range(N_CORES):
        er = slice(E_PER_CORE * c, E_PER_CORE * (c + 1))
        tr = slice(TC * c, TC * (c + 1))
        # xT[p, ko, t] = x[tr][t, 128*ko + p]
        xT = np.ascontiguousarray(
            x[tr].T.reshape(KH, 128, TC).transpose(1, 0, 2)
        )
        # wgu flat layout per expert/partition:
        #   [h, k, g|u, j] -> col h*12288 + k*768 + (0|384) + j
        wgc = wg[er].reshape(E_PER_CORE, KH, 128, 2, FH)  # e,k,p,h,j
        wuc = wu[er].reshape(E_PER_CORE, KH, 128, 2, FH)
        blk = np.stack([wgc, wuc], axis=4)  # e,k,p,h,gu,j
        wgu = np.ascontiguousarray(
            blk.transpose(0, 2, 3, 1, 4, 5).reshape(E_PER_CORE, 128, WGU_COLS)
        )
        # wd flat layout per expert/partition: per chunk of width w the
        # block is [k, w] (k-major), chunks concatenated.
        wdr = wdf[er].reshape(E_PER_CORE, KF, 128, H)  # e,k,p,col
        wd_rows = []
        for e in range(E_PER_CORE):
            colo = 0
            blocks = []
            for w in WD_SPLITS[e]:
                blocks.append(
                    wdr[e][:, :, colo : colo + w]
                    .transpose(1, 0, 2)
                    .reshape(128, KF * w)
                )
                colo += w
            wd_rows.append(np.concatenate(blocks, axis=1))
        wdl = np.ascontiguousarray(np.stack(wd_rows, axis=0))
        in_maps.append({"xT": xT, "ident": ident, "wgu": wgu, "wd": wdl})
    return in_maps


def kernel(expert_tokens, expert_tokens_count, gate_proj, up_proj, down_proj):
    in_maps = _make_in_maps(expert_tokens, gate_proj, up_proj, down_proj)
    results = _exec(in_maps)
    y = np.concatenate([results[c]["out"] for c in range(N_CORES)], axis=0)
    return np.asarray(y, dtype=np.float32)
